# revision 12
# baseline (speedup 1.0000x reference)
"""Bahdanau attention Trainium2 kernel.

Full-input contract: kernel(**inputs) takes the unsharded numpy inputs and
returns (context [B,E] f32, attn [B,L] f32), matching the reference.

Strategy: data-parallel over batch across 8 NeuronCores (B=32 -> 4 per core),
no collectives. Per core, for each batch row b:
  enc_t = enc @ W1^T       -- PE matmul, bf16 operands, fp32 PSUM accum
  comb  = tanh(enc_t + dec_t[b])  -- ACT, per-partition bias
  s     = v . comb         -- PE matmul with v as stationary [A,1]
  attn  = softmax(s + (mask-1)*1e30)  -- ACT exp with accum_out sum
  ctx   = attn^T @ enc     -- PE matmul over l, enc in natural layout

encoder_outputs reach SBUF in natural [l,e] layout (contiguous DMA with
fp32->bf16 cast); the e-major layout needed for the W1 matmul is produced
on-chip with PE transposes of 128x128 blocks.
"""

import numpy as np
from contextlib import ExitStack

import concourse.bass as bass
import concourse.tile as tile
from concourse import mybir
from concourse.bass_utils import run_bass_kernel_spmd
from concourse.masks import make_identity

F32 = mybir.dt.float32
BF16 = mybir.dt.bfloat16
I32 = mybir.dt.int32
AF = mybir.ActivationFunctionType
ALU = mybir.AluOpType

P = 128
N_CORES = 8
B_FULL, L_FULL, E_FULL, A_FULL, D_FULL = 32, 2048, 1024, 1024, 1024


def build_program(B_LOC=4, L=2048, E=1024, A=1024, D=1024, LC=512):
    """Emit the per-core SPMD program. All cores run this same program on
    their own batch shard."""
    assert L % LC == 0 and LC % P == 0
    N_LC = L // LC          # l-chunks per batch row
    TPC = LC // P           # 128-wide l-subtiles per chunk
    N_LT = L // P           # l-subtiles per batch row
    EC = E // P             # e chunks (contraction for W1 matmul)
    AT = A // P             # a tiles (output partition tiles)
    DC = D // P             # d chunks (contraction for W2 matmul)
    ECX = max(1, E // 512)  # 512-wide e chunks for context matmul
    ECW = min(E, 512)

    nc = bass.Bass()
    enc = nc.dram_tensor("enc", [B_LOC, L, E], F32, kind="ExternalInput")
    w1t = nc.dram_tensor("w1t", [E, A], F32, kind="ExternalInput")
    w2t = nc.dram_tensor("w2t", [D, A], F32, kind="ExternalInput")
    dect = nc.dram_tensor("dect", [D, B_LOC], F32, kind="ExternalInput")
    vt = nc.dram_tensor("vt", [A], F32, kind="ExternalInput")
    maskd = nc.dram_tensor("mask", [B_LOC, L], I32, kind="ExternalInput")
    ctx_out = nc.dram_tensor("ctx_out", [B_LOC, E], F32, kind="ExternalOutput")
    attn_out = nc.dram_tensor("attn_out", [B_LOC, L], F32, kind="ExternalOutput")
    attn_scr = nc.dram_tensor("attn_scr", [B_LOC, L], BF16)

    with tile.TileContext(nc) as tc:
        with ExitStack() as ctx:
            const = ctx.enter_context(tc.tile_pool(name="const", bufs=1))
            natp = ctx.enter_context(tc.tile_pool(name="natp", bufs=2))
            encp = ctx.enter_context(tc.tile_pool(name="encp", bufs=2))
            combp = ctx.enter_context(tc.tile_pool(name="combp", bufs=4))
            pmain = ctx.enter_context(tc.tile_pool(name="pmain", bufs=2, space="PSUM"))
            ptr = ctx.enter_context(tc.tile_pool(name="ptr", bufs=2, space="PSUM"))
            psmall = ctx.enter_context(tc.tile_pool(name="psmall", bufs=3, space="PSUM"))

            # ---- constants / weights ----
            w1t_sb = const.tile([P, EC, A], BF16)
            nc.gpsimd.dma_start(w1t_sb[:], w1t.rearrange("(c p) a -> p c a", p=P))
            w2t_sb = const.tile([P, DC, A], BF16)
            nc.gpsimd.dma_start(w2t_sb[:], w2t.rearrange("(c p) a -> p c a", p=P))
            dect_sb = const.tile([P, DC, B_LOC], BF16)
            nc.gpsimd.dma_start(dect_sb[:], dect.rearrange("(c p) b -> p c b", p=P))
            vt_sb = const.tile([P, AT], BF16)
            nc.gpsimd.dma_start(vt_sb[:], vt.rearrange("(c p) -> p c", p=P))
            # Warmup activation with zero dependencies: absorbs the one-time
            # ACT table-set load so no real tanh/exp carries it (walrus allows
            # at most 2 sync waits per instruction; the table load uses one).
            warm = const.tile([1, 2], F32)
            warm2 = const.tile([1, 2], F32)
            nc.gpsimd.memset(warm[:], 0.0)
            nc.scalar.activation(warm2[:], warm[:], AF.Tanh)
            ident = const.tile([P, P], BF16)
            make_identity(nc, ident)

            # Engines can only address SBUF starting at partition 0/32/64/96,
            # so per-batch-row rows live at partition b*32.
            R = lambda b: b * 32
            mask_i = const.tile([P, L], I32)
            maskb = const.tile([P, L], F32)
            for b in range(B_LOC):
                nc.sync.dma_start(mask_i[R(b):R(b) + 1, :], maskd[b:b + 1, :])
                nc.vector.tensor_copy(maskb[R(b):R(b) + 1, :], mask_i[R(b):R(b) + 1, :])
                nc.vector.tensor_scalar(
                    maskb[R(b):R(b) + 1, :], maskb[R(b):R(b) + 1, :],
                    1e30, -1e30, ALU.mult, ALU.add,
                )

            decb_sb = const.tile([P, AT, B_LOC], F32)
            scores_sb = const.tile([P, L], F32)
            probs_sb = const.tile([P, L], F32)
            probs_bf = const.tile([P, L], BF16)
            sumexp = const.tile([P, 1], F32)
            rsum = const.tile([P, 1], F32)
            ctx_sb = const.tile([P, E], F32)
            attnT_sb = const.tile([P, B_LOC, N_LT], BF16)

            # ---- dec_t[b,a] = decoder_hidden @ W2^T, laid out [a_part, b] ----
            for at in range(AT):
                ps_d = psmall.tile([P, B_LOC], F32, tag="small", name="ps_d")
                for dc in range(DC):
                    nc.tensor.matmul(
                        ps_d[:],
                        lhsT=w2t_sb[:, dc, at * P:(at + 1) * P],
                        rhs=dect_sb[:, dc, :],
                        start=(dc == 0),
                        stop=(dc == DC - 1),
                    )
                # Copy on the scalar engine: the tanh bias dependency then
                # stays same-engine (implicit FIFO order, no sem wait).
                nc.scalar.copy(decb_sb[:, at, :], ps_d[:])

            nat_tiles = {}

            def emit_chunk(b, lc):
                """Main compute for 512 l positions of batch row b."""
                if lc == 0:
                    nat_tiles[b] = natp.tile([P, N_LT, E], BF16, tag="nat", name="nat")
                nat = nat_tiles[b]
                lt0 = lc * TPC
                nc.gpsimd.dma_start(
                    nat[:, lt0:lt0 + TPC, :],
                    enc[b].rearrange("(t p) e -> p t e", p=P)[:, lt0:lt0 + TPC, :],
                )
                encT = encp.tile([P, EC, LC], BF16, tag="encT", name="encT")
                for ec in range(EC):
                    for t in range(TPC):
                        ps_t = ptr.tile([P, P], BF16, tag="ps_t", name="ps_t")
                        nc.tensor.transpose(
                            ps_t[:], nat[:, lt0 + t, ec * P:(ec + 1) * P], ident[:]
                        )
                        nc.vector.tensor_copy(encT[:, ec, t * P:(t + 1) * P], ps_t[:])

                ps_s = psmall.tile([1, LC], F32, tag="small", name="ps_s")
                pending_score = None
                for at in range(AT):
                    ps_m = pmain.tile([P, LC], F32, tag="ps_m", name="ps_m")
                    for ec in range(EC):
                        nc.tensor.matmul(
                            ps_m[:],
                            lhsT=w1t_sb[:, ec, at * P:(at + 1) * P],
                            rhs=encT[:, ec, :],
                            start=(ec == 0),
                            stop=(ec == EC - 1),
                        )
                    comb = combp.tile([P, LC], BF16, tag="comb", name="comb")
                    nc.scalar.activation(
                        comb[:], ps_m[:], AF.Tanh, bias=decb_sb[:, at, b:b + 1]
                    )
                    # Delay each v-dot matmul by one a-tile so the PE never
                    # waits on the ACT tanh that produces its rhs.
                    if pending_score is not None:
                        pat, pcomb = pending_score
                        nc.tensor.matmul(
                            ps_s[:], lhsT=vt_sb[:, pat:pat + 1], rhs=pcomb[:],
                            start=(pat == 0), stop=False, skip_group_check=True,
                        )
                    pending_score = (at, comb)
                pat, pcomb = pending_score
                nc.tensor.matmul(
                    ps_s[:], lhsT=vt_sb[:, pat:pat + 1], rhs=pcomb[:],
                    start=False, stop=True, skip_group_check=True,
                )
                nc.vector.tensor_copy(scores_sb[R(b):R(b) + 1, lc * LC:(lc + 1) * LC], ps_s[:])

            def emit_epilogue(b):
                """Softmax over L, attn output, and context for batch row b."""
                r = R(b)
                nc.vector.tensor_tensor(
                    scores_sb[r:r + 1, :], scores_sb[r:r + 1, :], maskb[r:r + 1, :],
                    ALU.add,
                )
                nc.scalar.activation(
                    probs_sb[r:r + 1, :], scores_sb[r:r + 1, :], AF.Exp,
                    accum_out=sumexp[r:r + 1, :],
                )
                nc.vector.reciprocal(rsum[r:r + 1, :], sumexp[r:r + 1, :])
                nc.vector.tensor_scalar_mul(
                    probs_sb[r:r + 1, :], probs_sb[r:r + 1, :], rsum[r:r + 1, :]
                )
                nc.sync.dma_start(attn_out[b:b + 1, :], probs_sb[r:r + 1, :])
                # Round-trip through DRAM to flip attn into [l_part, l_tile]
                # layout (xbar transpose needs a 2-byte dtype and a DRAM src).
                nc.vector.tensor_copy(probs_bf[r:r + 1, :], probs_sb[r:r + 1, :])
                nc.sync.dma_start(attn_scr[b:b + 1, :], probs_bf[r:r + 1, :])
                nc.sync.dma_start_transpose(
                    attnT_sb[:, b, :], attn_scr[b].rearrange("(o p) -> o p", p=P)
                )
                nat = nat_tiles.pop(b)
                for ecx in range(ECX):
                    ps_c = psmall.tile([1, ECW], F32, tag="small", name="ps_c")
                    for t in range(N_LT):
                        nc.tensor.matmul(
                            ps_c[:],
                            lhsT=attnT_sb[:, b, t:t + 1],
                            rhs=nat[:, t, ecx * ECW:(ecx + 1) * ECW],
                            start=(t == 0),
                            stop=(t == N_LT - 1),
                        )
                    nc.vector.tensor_copy(
                        ctx_sb[r:r + 1, ecx * ECW:(ecx + 1) * ECW], ps_c[:]
                    )
                nc.sync.dma_start(ctx_out[b:b + 1, :], ctx_sb[r:r + 1, :])

            # Epilogue for row b is emitted after the first chunk of row b+1 so
            # the PE has dense matmul work covering the softmax/DMA latency.
            pending_epi = None
            for b in range(B_LOC):
                for lc in range(N_LC):
                    emit_chunk(b, lc)
                    if pending_epi is not None:
                        emit_epilogue(pending_epi)
                        pending_epi = None
                pending_epi = b
            emit_epilogue(pending_epi)

    _split_excess_waits(nc)
    return nc


def _split_excess_waits(nc, max_waits=1):
    """Walrus codegen allows at most `max_waits` sync-wait commands per
    instruction, but Tile's sem assignment can emit more (notably the
    kernel-tail drain). Hoist the excess onto same-engine NoOps inserted
    immediately before the instruction — engine queues execute in FIFO
    order, so the semantics are identical."""
    k = 0
    for f in nc.m.functions:
        for bb in f.blocks:
            out = []
            for ins in bb.instructions:
                si = ins.sync_info
                if si is None:
                    out.append(ins)
                    continue
                waits = list(si.on_wait)
                updates = list(si.on_update)
                upd_ids = {u.id for u in updates}
                # A wait on a semaphore this instruction also updates costs an
                # extra sync command in walrus codegen — always hoist those.
                excess = [w for w in waits if w.id in upd_ids]
                keep = [w for w in waits if w.id not in upd_ids]
                if len(keep) > max_waits:
                    excess.extend(keep[:-max_waits])
                    keep = keep[-max_waits:]
                if not excess:
                    out.append(ins)
                    continue
                for w in excess:
                    nop = mybir.InstNoOp(name=f"I-waitsplit-{k}", ins=[], outs=[])
                    k += 1
                    nop.engine = ins.engine
                    nop.sync_info = mybir.SyncInfo(on_wait=[w], on_update=[])
                    out.append(nop)
                ins.sync_info = mybir.SyncInfo(on_wait=keep, on_update=updates)
                out.append(ins)
            bb.instructions[:] = out


_PROGRAM_CACHE = {}


def _get_program():
    key = "full"
    if key not in _PROGRAM_CACHE:
        _PROGRAM_CACHE[key] = build_program()
    return _PROGRAM_CACHE[key]


LAST_RESULTS = None


def kernel(encoder_outputs, decoder_hidden, mask, W1, W2, v, _trace=False):
    global LAST_RESULTS
    enc = np.ascontiguousarray(encoder_outputs, dtype=np.float32)
    dec = np.ascontiguousarray(decoder_hidden, dtype=np.float32)
    mask = np.ascontiguousarray(mask, dtype=np.int32)
    w1t = np.ascontiguousarray(np.asarray(W1, dtype=np.float32).T)
    w2t = np.ascontiguousarray(np.asarray(W2, dtype=np.float32).T)
    vt = np.ascontiguousarray(np.asarray(v, dtype=np.float32).reshape(-1))

    B = enc.shape[0]
    b_loc = B // N_CORES
    nc = _get_program()

    in_maps = []
    for i in range(N_CORES):
        sl = slice(i * b_loc, (i + 1) * b_loc)
        in_maps.append({
            "enc": enc[sl],
            "w1t": w1t,
            "w2t": w2t,
            "dect": np.ascontiguousarray(dec[sl].T),
            "vt": vt,
            "mask": mask[sl],
        })

    res = run_bass_kernel_spmd(
        nc, in_maps, core_ids=list(range(N_CORES)), trace=_trace
    )
    LAST_RESULTS = res
    ctx = np.concatenate([r["ctx_out"] for r in res.results], axis=0)
    attn = np.concatenate([r["attn_out"] for r in res.results], axis=0)
    return ctx.astype(np.float32), attn.astype(np.float32)


# revision 16
# speedup vs baseline: 1.0142x; 1.0142x over previous
"""Bahdanau attention Trainium2 kernel.

Full-input contract: kernel(**inputs) takes the unsharded numpy inputs and
returns (context [B,E] f32, attn [B,L] f32), matching the reference.

Strategy: data-parallel over batch across 8 NeuronCores (B=32 -> 4 per core),
no collectives. Per core, for each batch row b:
  enc_t = enc @ W1^T       -- PE matmul, bf16 operands, fp32 PSUM accum
  comb  = tanh(enc_t + dec_t[b])  -- ACT, per-partition bias
  s     = v . comb         -- PE matmul with v as stationary [A,1]
  attn  = softmax(s + (mask-1)*1e30)  -- ACT exp with accum_out sum
  ctx   = attn^T @ enc     -- PE matmul over l, enc in natural layout

encoder_outputs reach SBUF in natural [l,e] layout (contiguous DMA with
fp32->bf16 cast); the e-major layout needed for the W1 matmul is produced
on-chip with PE transposes of 128x128 blocks.
"""

import numpy as np
from contextlib import ExitStack

import concourse.bass as bass
import concourse.tile as tile
from concourse import mybir
from concourse.bass_utils import run_bass_kernel_spmd
from concourse.masks import make_identity

F32 = mybir.dt.float32
BF16 = mybir.dt.bfloat16
I32 = mybir.dt.int32
AF = mybir.ActivationFunctionType
ALU = mybir.AluOpType

P = 128
N_CORES = 8
B_FULL, L_FULL, E_FULL, A_FULL, D_FULL = 32, 2048, 1024, 1024, 1024


def build_program(B_LOC=4, L=2048, E=1024, A=1024, D=1024, LC=512):
    """Emit the per-core SPMD program. All cores run this same program on
    their own batch shard."""
    assert L % LC == 0 and LC % P == 0
    N_LC = L // LC          # l-chunks per batch row
    TPC = LC // P           # 128-wide l-subtiles per chunk
    N_LT = L // P           # l-subtiles per batch row
    EC = E // P             # e chunks (contraction for W1 matmul)
    AT = A // P             # a tiles (output partition tiles)
    DC = D // P             # d chunks (contraction for W2 matmul)
    ECX = max(1, E // 512)  # 512-wide e chunks for context matmul
    ECW = min(E, 512)

    nc = bass.Bass()
    enc = nc.dram_tensor("enc", [B_LOC, L, E], F32, kind="ExternalInput")
    w1t = nc.dram_tensor("w1t", [E, A], F32, kind="ExternalInput")
    w2t = nc.dram_tensor("w2t", [D, A], F32, kind="ExternalInput")
    dect = nc.dram_tensor("dect", [D, B_LOC], F32, kind="ExternalInput")
    vt = nc.dram_tensor("vt", [A], F32, kind="ExternalInput")
    maskd = nc.dram_tensor("mask", [B_LOC, L], I32, kind="ExternalInput")
    ctx_out = nc.dram_tensor("ctx_out", [B_LOC, E], F32, kind="ExternalOutput")
    attn_out = nc.dram_tensor("attn_out", [B_LOC, L], F32, kind="ExternalOutput")
    attn_scr = nc.dram_tensor("attn_scr", [B_LOC, L], BF16)

    with tile.TileContext(nc) as tc:
        with ExitStack() as ctx:
            const = ctx.enter_context(tc.tile_pool(name="const", bufs=1))
            natp = ctx.enter_context(tc.tile_pool(name="natp", bufs=2))
            encp = ctx.enter_context(tc.tile_pool(name="encp", bufs=2))
            combp = ctx.enter_context(tc.tile_pool(name="combp", bufs=4))
            pmain = ctx.enter_context(tc.tile_pool(name="pmain", bufs=2, space="PSUM"))
            ptr = ctx.enter_context(tc.tile_pool(name="ptr", bufs=2, space="PSUM"))
            psmall = ctx.enter_context(tc.tile_pool(name="psmall", bufs=3, space="PSUM"))

            # ---- constants / weights ----
            w1t_sb = const.tile([P, EC, A], BF16)
            nc.gpsimd.dma_start(w1t_sb[:], w1t.rearrange("(c p) a -> p c a", p=P))
            w2t_sb = const.tile([P, DC, A], BF16)
            nc.gpsimd.dma_start(w2t_sb[:], w2t.rearrange("(c p) a -> p c a", p=P))
            dect_sb = const.tile([P, DC, B_LOC], BF16)
            nc.gpsimd.dma_start(dect_sb[:], dect.rearrange("(c p) b -> p c b", p=P))
            vt_sb = const.tile([P, AT], BF16)
            nc.gpsimd.dma_start(vt_sb[:], vt.rearrange("(c p) -> p c", p=P))
            # Warmup activation with zero dependencies: absorbs the one-time
            # ACT table-set load so no real tanh/exp carries it (walrus allows
            # at most 2 sync waits per instruction; the table load uses one).
            warm = const.tile([1, 2], F32)
            warm2 = const.tile([1, 2], F32)
            nc.gpsimd.memset(warm[:], 0.0)
            nc.scalar.activation(warm2[:], warm[:], AF.Tanh)
            ident = const.tile([P, P], BF16)
            make_identity(nc, ident)

            # Engines can only address SBUF starting at partition 0/32/64/96,
            # so per-batch-row rows live at partition b*32.
            R = lambda b: b * 32
            mask_i = const.tile([P, L], I32)
            maskb = const.tile([P, L], F32)
            for b in range(B_LOC):
                nc.sync.dma_start(mask_i[R(b):R(b) + 1, :], maskd[b:b + 1, :])
                nc.vector.tensor_copy(maskb[R(b):R(b) + 1, :], mask_i[R(b):R(b) + 1, :])
                nc.vector.tensor_scalar(
                    maskb[R(b):R(b) + 1, :], maskb[R(b):R(b) + 1, :],
                    1e30, -1e30, ALU.mult, ALU.add,
                )

            decb_sb = const.tile([P, AT, B_LOC], F32)
            scores_sb = const.tile([P, L], F32)
            probs_sb = const.tile([P, L], F32)
            probs_bf = const.tile([P, L], BF16)
            sumc = const.tile([P, N_LC], F32)
            sumexp = const.tile([P, 1], F32)
            rsum = const.tile([P, 1], F32)
            ctx_sb = const.tile([P, E], F32)
            attnT_sb = const.tile([P, B_LOC, N_LT], BF16)

            # ---- dec_t[b,a] = decoder_hidden @ W2^T, laid out [a_part, b] ----
            for at in range(AT):
                ps_d = psmall.tile([P, B_LOC], F32, tag="small", name="ps_d")
                for dc in range(DC):
                    nc.tensor.matmul(
                        ps_d[:],
                        lhsT=w2t_sb[:, dc, at * P:(at + 1) * P],
                        rhs=dect_sb[:, dc, :],
                        start=(dc == 0),
                        stop=(dc == DC - 1),
                    )
                # Copy on the scalar engine: the tanh bias dependency then
                # stays same-engine (implicit FIFO order, no sem wait).
                nc.scalar.copy(decb_sb[:, at, :], ps_d[:])

            nat_tiles = {}

            def emit_chunk(b, lc):
                """Main compute for 512 l positions of batch row b."""
                if lc == 0:
                    nat_tiles[b] = natp.tile([P, N_LT, E], BF16, tag="nat", name="nat")
                nat = nat_tiles[b]
                lt0 = lc * TPC
                nc.gpsimd.dma_start(
                    nat[:, lt0:lt0 + TPC, :],
                    enc[b].rearrange("(t p) e -> p t e", p=P)[:, lt0:lt0 + TPC, :],
                )
                encT = encp.tile([P, EC, LC], BF16, tag="encT", name="encT")
                for ec in range(EC):
                    for t in range(TPC):
                        ps_t = ptr.tile([P, P], BF16, tag="ps_t", name="ps_t")
                        nc.tensor.transpose(
                            ps_t[:], nat[:, lt0 + t, ec * P:(ec + 1) * P], ident[:]
                        )
                        nc.vector.tensor_copy(encT[:, ec, t * P:(t + 1) * P], ps_t[:])

                ps_s = psmall.tile([1, LC], F32, tag="small", name="ps_s")
                pending_score = None
                for at in range(AT):
                    ps_m = pmain.tile([P, LC], F32, tag="ps_m", name="ps_m")
                    for ec in range(EC):
                        nc.tensor.matmul(
                            ps_m[:],
                            lhsT=w1t_sb[:, ec, at * P:(at + 1) * P],
                            rhs=encT[:, ec, :],
                            start=(ec == 0),
                            stop=(ec == EC - 1),
                        )
                    comb = combp.tile([P, LC], BF16, tag="comb", name="comb")
                    nc.scalar.activation(
                        comb[:], ps_m[:], AF.Tanh, bias=decb_sb[:, at, b:b + 1]
                    )
                    # Delay each v-dot matmul by one a-tile so the PE never
                    # waits on the ACT tanh that produces its rhs.
                    if pending_score is not None:
                        pat, pcomb = pending_score
                        nc.tensor.matmul(
                            ps_s[:], lhsT=vt_sb[:, pat:pat + 1], rhs=pcomb[:],
                            start=(pat == 0), stop=False, skip_group_check=True,
                        )
                    pending_score = (at, comb)
                pat, pcomb = pending_score
                nc.tensor.matmul(
                    ps_s[:], lhsT=vt_sb[:, pat:pat + 1], rhs=pcomb[:],
                    start=False, stop=True, skip_group_check=True,
                )
                # Per-chunk softmax front half, pipelined into the main loop:
                # mask, exp (with per-chunk sum), bf16 cast, and the DRAM
                # staging write all happen while later chunks still matmul.
                r = R(b)
                sl = slice(lc * LC, (lc + 1) * LC)
                nc.vector.tensor_copy(scores_sb[r:r + 1, sl], ps_s[:])
                nc.vector.tensor_tensor(
                    scores_sb[r:r + 1, sl], scores_sb[r:r + 1, sl],
                    maskb[r:r + 1, sl], ALU.add,
                )
                nc.scalar.activation(
                    probs_sb[r:r + 1, sl], scores_sb[r:r + 1, sl], AF.Exp,
                    accum_out=sumc[r:r + 1, lc:lc + 1],
                )
                nc.vector.tensor_copy(probs_bf[r:r + 1, sl], probs_sb[r:r + 1, sl])
                nc.sync.dma_start(attn_scr[b:b + 1, sl], probs_bf[r:r + 1, sl])

            def emit_epilogue(b):
                """Context (from unnormalized exp weights) + normalized attn
                output for batch row b. The exp/cast/staging already ran
                per-chunk; the context matmul only needs the xbar transpose."""
                r = R(b)
                # attnT holds UNNORMALIZED exp(s) bf16; normalization is folded
                # into the PSUM->SBUF copy (ctx) and a DVE scale (attn out).
                nc.sync.dma_start_transpose(
                    attnT_sb[:, b, :], attn_scr[b].rearrange("(o p) -> o p", p=P)
                )
                nc.vector.reduce_sum(
                    sumexp[r:r + 1, :], sumc[r:r + 1, :], axis=mybir.AxisListType.X
                )
                nc.vector.reciprocal(rsum[r:r + 1, :], sumexp[r:r + 1, :])
                nat = nat_tiles.pop(b)
                for ecx in range(ECX):
                    ps_c = psmall.tile([1, ECW], F32, tag="small", name="ps_c")
                    for t in range(N_LT):
                        nc.tensor.matmul(
                            ps_c[:],
                            lhsT=attnT_sb[:, b, t:t + 1],
                            rhs=nat[:, t, ecx * ECW:(ecx + 1) * ECW],
                            start=(t == 0),
                            stop=(t == N_LT - 1),
                        )
                    nc.vector.tensor_scalar_mul(
                        ctx_sb[r:r + 1, ecx * ECW:(ecx + 1) * ECW], ps_c[:],
                        rsum[r:r + 1, :],
                    )
                nc.sync.dma_start(ctx_out[b:b + 1, :], ctx_sb[r:r + 1, :])
                # attn output (off the critical path).
                nc.vector.tensor_scalar_mul(
                    probs_sb[r:r + 1, :], probs_sb[r:r + 1, :], rsum[r:r + 1, :]
                )
                nc.sync.dma_start(attn_out[b:b + 1, :], probs_sb[r:r + 1, :])

            # Epilogue for row b is emitted after the first chunk of row b+1 so
            # the PE has dense matmul work covering the softmax/DMA latency.
            pending_epi = None
            for b in range(B_LOC):
                for lc in range(N_LC):
                    emit_chunk(b, lc)
                    if pending_epi is not None:
                        emit_epilogue(pending_epi)
                        pending_epi = None
                pending_epi = b
            emit_epilogue(pending_epi)

    _split_excess_waits(nc)
    return nc


def _split_excess_waits(nc, max_waits=1):
    """Walrus codegen allows at most `max_waits` sync-wait commands per
    instruction, but Tile's sem assignment can emit more (notably the
    kernel-tail drain). Hoist the excess onto same-engine NoOps inserted
    immediately before the instruction — engine queues execute in FIFO
    order, so the semantics are identical."""
    k = 0
    for f in nc.m.functions:
        for bb in f.blocks:
            out = []
            for ins in bb.instructions:
                si = ins.sync_info
                if si is None:
                    out.append(ins)
                    continue
                waits = list(si.on_wait)
                updates = list(si.on_update)
                upd_ids = {u.id for u in updates}
                # A wait on a semaphore this instruction also updates costs an
                # extra sync command in walrus codegen — always hoist those.
                excess = [w for w in waits if w.id in upd_ids]
                keep = [w for w in waits if w.id not in upd_ids]
                if len(keep) > max_waits:
                    excess.extend(keep[:-max_waits])
                    keep = keep[-max_waits:]
                if not excess:
                    out.append(ins)
                    continue
                for w in excess:
                    nop = mybir.InstNoOp(name=f"I-waitsplit-{k}", ins=[], outs=[])
                    k += 1
                    nop.engine = ins.engine
                    nop.sync_info = mybir.SyncInfo(on_wait=[w], on_update=[])
                    nc.register_instruction(nop, overwrite=True)
                    out.append(nop)
                ins.sync_info = mybir.SyncInfo(on_wait=keep, on_update=updates)
                out.append(ins)
            bb.instructions[:] = out


_PROGRAM_CACHE = {}


def _get_program():
    key = "full"
    if key not in _PROGRAM_CACHE:
        _PROGRAM_CACHE[key] = build_program()
    return _PROGRAM_CACHE[key]


LAST_RESULTS = None


def kernel(encoder_outputs, decoder_hidden, mask, W1, W2, v, _trace=False):
    global LAST_RESULTS
    enc = np.ascontiguousarray(encoder_outputs, dtype=np.float32)
    dec = np.ascontiguousarray(decoder_hidden, dtype=np.float32)
    mask = np.ascontiguousarray(mask, dtype=np.int32)
    w1t = np.ascontiguousarray(np.asarray(W1, dtype=np.float32).T)
    w2t = np.ascontiguousarray(np.asarray(W2, dtype=np.float32).T)
    vt = np.ascontiguousarray(np.asarray(v, dtype=np.float32).reshape(-1))

    B = enc.shape[0]
    b_loc = B // N_CORES
    nc = _get_program()

    in_maps = []
    for i in range(N_CORES):
        sl = slice(i * b_loc, (i + 1) * b_loc)
        in_maps.append({
            "enc": enc[sl],
            "w1t": w1t,
            "w2t": w2t,
            "dect": np.ascontiguousarray(dec[sl].T),
            "vt": vt,
            "mask": mask[sl],
        })

    res = run_bass_kernel_spmd(
        nc, in_maps, core_ids=list(range(N_CORES)), trace=_trace
    )
    LAST_RESULTS = res
    ctx = np.concatenate([r["ctx_out"] for r in res.results], axis=0)
    attn = np.concatenate([r["attn_out"] for r in res.results], axis=0)
    return ctx.astype(np.float32), attn.astype(np.float32)


# revision 18
# speedup vs baseline: 1.0175x; 1.0033x over previous
"""Bahdanau attention Trainium2 kernel.

Full-input contract: kernel(**inputs) takes the unsharded numpy inputs and
returns (context [B,E] f32, attn [B,L] f32), matching the reference.

Strategy: data-parallel over batch across 8 NeuronCores (B=32 -> 4 per core),
no collectives. Per core, for each batch row b:
  enc_t = enc @ W1^T       -- PE matmul, bf16 operands, fp32 PSUM accum
  comb  = tanh(enc_t + dec_t[b])  -- ACT, per-partition bias
  s     = v . comb         -- PE matmul with v as stationary [A,1]
  attn  = softmax(s + (mask-1)*1e30)  -- ACT exp with accum_out sum
  ctx   = attn^T @ enc     -- PE matmul over l, enc in natural layout

encoder_outputs reach SBUF in natural [l,e] layout (contiguous DMA with
fp32->bf16 cast); the e-major layout needed for the W1 matmul is produced
on-chip with PE transposes of 128x128 blocks.
"""

import numpy as np
from contextlib import ExitStack

import concourse.bass as bass
import concourse.tile as tile
from concourse import mybir
from concourse.bass_utils import run_bass_kernel_spmd
from concourse.masks import make_identity

F32 = mybir.dt.float32
BF16 = mybir.dt.bfloat16
I32 = mybir.dt.int32
AF = mybir.ActivationFunctionType
ALU = mybir.AluOpType

P = 128
N_CORES = 8
B_FULL, L_FULL, E_FULL, A_FULL, D_FULL = 32, 2048, 1024, 1024, 1024


def build_program(B_LOC=4, L=2048, E=1024, A=1024, D=1024, LC=512):
    """Emit the per-core SPMD program. All cores run this same program on
    their own batch shard."""
    assert L % LC == 0 and LC % P == 0
    N_LC = L // LC          # l-chunks per batch row
    TPC = LC // P           # 128-wide l-subtiles per chunk
    N_LT = L // P           # l-subtiles per batch row
    EC = E // P             # e chunks (contraction for W1 matmul)
    AT = A // P             # a tiles (output partition tiles)
    DC = D // P             # d chunks (contraction for W2 matmul)
    ECX = max(1, E // 512)  # 512-wide e chunks for context matmul
    ECW = min(E, 512)

    nc = bass.Bass()
    enc = nc.dram_tensor("enc", [B_LOC, L, E], F32, kind="ExternalInput")
    w1t = nc.dram_tensor("w1t", [E, A], F32, kind="ExternalInput")
    w2t = nc.dram_tensor("w2t", [D, A], F32, kind="ExternalInput")
    dect = nc.dram_tensor("dect", [D, B_LOC], F32, kind="ExternalInput")
    vt = nc.dram_tensor("vt", [A], F32, kind="ExternalInput")
    maskd = nc.dram_tensor("mask", [B_LOC, L], I32, kind="ExternalInput")
    ctx_out = nc.dram_tensor("ctx_out", [B_LOC, E], F32, kind="ExternalOutput")
    attn_out = nc.dram_tensor("attn_out", [B_LOC, L], F32, kind="ExternalOutput")
    attn_scr = nc.dram_tensor("attn_scr", [B_LOC, L], BF16)
    encbf = nc.dram_tensor("encbf", [B_LOC, L, E], BF16)

    with tile.TileContext(nc) as tc:
        with ExitStack() as ctx:
            const = ctx.enter_context(tc.tile_pool(name="const", bufs=1))
            natp = ctx.enter_context(tc.tile_pool(name="natp", bufs=2))
            encp = ctx.enter_context(tc.tile_pool(name="encp", bufs=2))
            combp = ctx.enter_context(tc.tile_pool(name="combp", bufs=4))
            pmain = ctx.enter_context(tc.tile_pool(name="pmain", bufs=3, space="PSUM"))
            psmall = ctx.enter_context(tc.tile_pool(name="psmall", bufs=3, space="PSUM"))

            # ---- constants / weights ----
            w1t_sb = const.tile([P, EC, A], BF16)
            nc.gpsimd.dma_start(w1t_sb[:], w1t.rearrange("(c p) a -> p c a", p=P))
            w2t_sb = const.tile([P, DC, A], BF16)
            nc.gpsimd.dma_start(w2t_sb[:], w2t.rearrange("(c p) a -> p c a", p=P))
            dect_sb = const.tile([P, DC, B_LOC], BF16)
            nc.gpsimd.dma_start(dect_sb[:], dect.rearrange("(c p) b -> p c b", p=P))
            vt_sb = const.tile([P, AT], BF16)
            nc.gpsimd.dma_start(vt_sb[:], vt.rearrange("(c p) -> p c", p=P))
            # Warmup activation: absorbs the one-time ACT table-set load so no
            # real tanh/exp carries it (walrus allows at most 2 sync waits per
            # instruction; the table load uses one).
            warm = const.tile([1, 2], F32)
            warm2 = const.tile([1, 2], F32)
            nc.gpsimd.memset(warm[:], 0.0)
            nc.scalar.activation(warm2[:], warm[:], AF.Tanh)

            # Engines can only address SBUF starting at partition 0/32/64/96,
            # so per-batch-row rows live at partition b*32.
            R = lambda b: b * 32
            mask_i = const.tile([P, L], I32)
            maskb = const.tile([P, L], F32)
            for b in range(B_LOC):
                nc.sync.dma_start(mask_i[R(b):R(b) + 1, :], maskd[b:b + 1, :])
                nc.vector.tensor_copy(maskb[R(b):R(b) + 1, :], mask_i[R(b):R(b) + 1, :])
                nc.vector.tensor_scalar(
                    maskb[R(b):R(b) + 1, :], maskb[R(b):R(b) + 1, :],
                    1e30, -1e30, ALU.mult, ALU.add,
                )

            decb_sb = const.tile([P, AT, B_LOC], F32)
            scores_sb = const.tile([P, L], F32)
            probs_sb = const.tile([P, L], F32)
            probs_bf = const.tile([P, L], BF16)
            sumc = const.tile([P, N_LC], F32)
            sumexp = const.tile([P, 1], F32)
            rsum = const.tile([P, 1], F32)
            ctx_sb = const.tile([P, E], F32)
            attnT_sb = const.tile([P, B_LOC, N_LT], BF16)

            # ---- dec_t[b,a] = decoder_hidden @ W2^T, laid out [a_part, b] ----
            for at in range(AT):
                ps_d = psmall.tile([P, B_LOC], F32, tag="small", name="ps_d")
                for dc in range(DC):
                    nc.tensor.matmul(
                        ps_d[:],
                        lhsT=w2t_sb[:, dc, at * P:(at + 1) * P],
                        rhs=dect_sb[:, dc, :],
                        start=(dc == 0),
                        stop=(dc == DC - 1),
                    )
                # Copy on the scalar engine: the tanh bias dependency then
                # stays same-engine (implicit FIFO order, no sem wait).
                nc.scalar.copy(decb_sb[:, at, :], ps_d[:])

            # ---- chunk pipeline ------------------------------------------
            # chunks are (b, lc) pairs, globally indexed. Per-queue streams:
            #   gpsimd : nat cast loads only, prefetched 2 chunks ahead so the
            #            tiny attnT gathers never head-of-line block them
            #   sync   : scratch stage -> 8 xbar transposes (1 chunk ahead),
            #            then the chunk's attn staging write + attnT gather
            #   PE     : main matmuls -> (prev chunk's ctx matmuls) -> score
            #   ACT    : tanh with dec_t bias, per-chunk exp
            chunks = [(b, lc) for b in range(B_LOC) for lc in range(N_LC)]
            nat_tiles = {}
            encT_tiles = {}

            def emit_natdma(i):
                b, lc = chunks[i]
                if lc == 0:
                    nat_tiles[b] = natp.tile([P, N_LT, E], BF16, tag="nat", name="nat")
                lt0 = lc * TPC
                nc.gpsimd.dma_start(
                    nat_tiles[b][:, lt0:lt0 + TPC, :],
                    enc[b].rearrange("(t p) e -> p t e", p=P)[:, lt0:lt0 + TPC, :],
                )

            def emit_stage_xbar(i):
                b, lc = chunks[i]
                lt0 = lc * TPC
                nc.sync.dma_start(
                    encbf[b].rearrange("(t p) e -> p t e", p=P)[:, lt0:lt0 + TPC, :],
                    nat_tiles[b][:, lt0:lt0 + TPC, :],
                )
                encT = encp.tile([P, EC, LC], BF16, tag="encT", name="encT")
                encT_tiles[i] = encT
                for ec in range(EC):
                    nc.sync.dma_start_transpose(
                        encT[:, ec, :],
                        encbf[b, lc * LC:(lc + 1) * LC, ec * P:(ec + 1) * P],
                    )

            def emit_main(i):
                """Main matmuls + tanh + all-but-last score matmul."""
                b, lc = chunks[i]
                encT = encT_tiles[i]
                ps_s = psmall.tile([1, LC], F32, tag="small", name="ps_s")
                pending_score = None
                for at in range(AT):
                    ps_m = pmain.tile([P, LC], F32, tag="ps_m", name="ps_m")
                    for ec in range(EC):
                        nc.tensor.matmul(
                            ps_m[:],
                            lhsT=w1t_sb[:, ec, at * P:(at + 1) * P],
                            rhs=encT[:, ec, :],
                            start=(ec == 0),
                            stop=(ec == EC - 1),
                        )
                    comb = combp.tile([P, LC], BF16, tag="comb", name="comb")
                    nc.scalar.activation(
                        comb[:], ps_m[:], AF.Tanh, bias=decb_sb[:, at, b:b + 1]
                    )
                    # Delay each v-dot matmul by one a-tile so the PE never
                    # waits on the ACT tanh that produces its rhs.
                    if pending_score is not None:
                        pat, pcomb = pending_score
                        nc.tensor.matmul(
                            ps_s[:], lhsT=vt_sb[:, pat:pat + 1], rhs=pcomb[:],
                            start=(pat == 0), stop=False, skip_group_check=True,
                        )
                    pending_score = (at, comb)
                return ps_s, pending_score

            def emit_score_tail(i, ps_s, pending_score):
                """Last score matmul + per-chunk softmax front half."""
                b, lc = chunks[i]
                pat, pcomb = pending_score
                nc.tensor.matmul(
                    ps_s[:], lhsT=vt_sb[:, pat:pat + 1], rhs=pcomb[:],
                    start=False, stop=True, skip_group_check=True,
                )
                r = R(b)
                sl = slice(lc * LC, (lc + 1) * LC)
                nc.vector.tensor_copy(scores_sb[r:r + 1, sl], ps_s[:])
                nc.vector.tensor_tensor(
                    scores_sb[r:r + 1, sl], scores_sb[r:r + 1, sl],
                    maskb[r:r + 1, sl], ALU.add,
                )
                nc.scalar.activation(
                    probs_sb[r:r + 1, sl], scores_sb[r:r + 1, sl], AF.Exp,
                    accum_out=sumc[r:r + 1, lc:lc + 1],
                )
                nc.vector.tensor_copy(probs_bf[r:r + 1, sl], probs_sb[r:r + 1, sl])
                nc.sync.dma_start(attn_scr[b:b + 1, sl], probs_bf[r:r + 1, sl])

            def emit_epilogue(b):
                """Context (from unnormalized exp weights) + normalized attn
                output for batch row b."""
                r = R(b)
                nc.sync.dma_start_transpose(
                    attnT_sb[:, b, :], attn_scr[b].rearrange("(o p) -> o p", p=P)
                )
                nc.vector.reduce_sum(
                    sumexp[r:r + 1, :], sumc[r:r + 1, :], axis=mybir.AxisListType.X
                )
                nc.vector.reciprocal(rsum[r:r + 1, :], sumexp[r:r + 1, :])
                nat = nat_tiles.pop(b)
                for ecx in range(ECX):
                    ps_c = psmall.tile([1, ECW], F32, tag="small", name="ps_c")
                    for t in range(N_LT):
                        nc.tensor.matmul(
                            ps_c[:],
                            lhsT=attnT_sb[:, b, t:t + 1],
                            rhs=nat[:, t, ecx * ECW:(ecx + 1) * ECW],
                            start=(t == 0),
                            stop=(t == N_LT - 1),
                            skip_group_check=True,
                        )
                    nc.vector.tensor_scalar_mul(
                        ctx_sb[r:r + 1, ecx * ECW:(ecx + 1) * ECW], ps_c[:],
                        rsum[r:r + 1, :],
                    )
                nc.sync.dma_start(ctx_out[b:b + 1, :], ctx_sb[r:r + 1, :])
                nc.vector.tensor_scalar_mul(
                    probs_sb[r:r + 1, :], probs_sb[r:r + 1, :], rsum[r:r + 1, :]
                )
                nc.sync.dma_start(attn_out[b:b + 1, :], probs_sb[r:r + 1, :])

            # prologue: fill the prefetch pipeline
            emit_natdma(0)
            emit_natdma(1)
            emit_stage_xbar(0)
            pending_epi = None
            for i in range(len(chunks)):
                b, lc = chunks[i]
                if i + 2 < len(chunks):
                    emit_natdma(i + 2)
                if i + 1 < len(chunks):
                    emit_stage_xbar(i + 1)
                ps_s, pending_score = emit_main(i)
                if pending_epi is not None:
                    emit_epilogue(pending_epi)
                    pending_epi = None
                emit_score_tail(i, ps_s, pending_score)
                encT_tiles.pop(i, None)
                if lc == N_LC - 1:
                    pending_epi = b
            emit_epilogue(pending_epi)

    _split_excess_waits(nc)
    return nc


def _split_excess_waits(nc, max_waits=1):
    """Walrus codegen allows at most `max_waits` sync-wait commands per
    instruction, but Tile's sem assignment can emit more (notably the
    kernel-tail drain). Hoist the excess onto same-engine NoOps inserted
    immediately before the instruction — engine queues execute in FIFO
    order, so the semantics are identical."""
    k = 0
    for f in nc.m.functions:
        for bb in f.blocks:
            out = []
            for ins in bb.instructions:
                si = ins.sync_info
                if si is None:
                    out.append(ins)
                    continue
                waits = list(si.on_wait)
                updates = list(si.on_update)
                upd_ids = {u.id for u in updates}
                # A wait on a semaphore this instruction also updates costs an
                # extra sync command in walrus codegen — always hoist those.
                excess = [w for w in waits if w.id in upd_ids]
                keep = [w for w in waits if w.id not in upd_ids]
                if len(keep) > max_waits:
                    excess.extend(keep[:-max_waits])
                    keep = keep[-max_waits:]
                if not excess:
                    out.append(ins)
                    continue
                for w in excess:
                    nop = mybir.InstNoOp(name=f"I-waitsplit-{k}", ins=[], outs=[])
                    k += 1
                    nop.engine = ins.engine
                    nop.sync_info = mybir.SyncInfo(on_wait=[w], on_update=[])
                    nc.register_instruction(nop, overwrite=True)
                    out.append(nop)
                ins.sync_info = mybir.SyncInfo(on_wait=keep, on_update=updates)
                out.append(ins)
            bb.instructions[:] = out


_PROGRAM_CACHE = {}


def _get_program():
    key = "full"
    if key not in _PROGRAM_CACHE:
        _PROGRAM_CACHE[key] = build_program()
    return _PROGRAM_CACHE[key]


LAST_RESULTS = None


def kernel(encoder_outputs, decoder_hidden, mask, W1, W2, v, _trace=False):
    global LAST_RESULTS
    enc = np.ascontiguousarray(encoder_outputs, dtype=np.float32)
    dec = np.ascontiguousarray(decoder_hidden, dtype=np.float32)
    mask = np.ascontiguousarray(mask, dtype=np.int32)
    w1t = np.ascontiguousarray(np.asarray(W1, dtype=np.float32).T)
    w2t = np.ascontiguousarray(np.asarray(W2, dtype=np.float32).T)
    vt = np.ascontiguousarray(np.asarray(v, dtype=np.float32).reshape(-1))

    B = enc.shape[0]
    b_loc = B // N_CORES
    nc = _get_program()

    in_maps = []
    for i in range(N_CORES):
        sl = slice(i * b_loc, (i + 1) * b_loc)
        in_maps.append({
            "enc": enc[sl],
            "w1t": w1t,
            "w2t": w2t,
            "dect": np.ascontiguousarray(dec[sl].T),
            "vt": vt,
            "mask": mask[sl],
        })

    res = run_bass_kernel_spmd(
        nc, in_maps, core_ids=list(range(N_CORES)), trace=_trace
    )
    LAST_RESULTS = res
    ctx = np.concatenate([r["ctx_out"] for r in res.results], axis=0)
    attn = np.concatenate([r["attn_out"] for r in res.results], axis=0)
    return ctx.astype(np.float32), attn.astype(np.float32)
